# revision 77
# baseline (speedup 1.0000x reference)
"""Trainium2 Bass kernel for GQA causal attention (nn_Attention_89816356094768).

Math (per reference):
  q = x @ wq.T + bq ; k = x @ wk.T + bk ; v = x @ wv.T + bv
  RoPE on q, k; S = q @ k.T * D**-0.5 with causal mask; P = softmax(S)
  out = (P @ v) reassembled over heads @ wo.T

Sharding: tensor-parallel over heads across 8 cores. Core c owns q heads
(2c, 2c+1) and kv head c//4. Each core computes its two heads' attention and
a row-parallel partial of the output projection; the host sums the 8 partials
(fp16 partials, summed in fp32).

Perf structure (v4):
- q/k/v and output projections run as fp8e4m3 DoubleRow matmuls (contraction
  pairs packed on the partition dim, 4x bf16 throughput per k-tile in the
  cost model). Precision is kept with a hi+lo split of both operands and 3
  cross terms (hi*hi + hi*lo + lo*hi) = 0.75x the bf16 PE cycles. Weights are
  pre-scaled by 64 on the host (w std 0.012 is below e4m3's normal range);
  the 1/64 is folded into the lin copy / rope products. The attention output
  is scaled by 8 via the rowsum ones matrix (host-side ones/8) before its
  fp8 hi/lo split on the Pool engine, and the o-projection PSUM is divided
  by 512 in the PSUM->fp16 copies (split DVE/ACT).
- Attention (scores/softmax/PV/rowsum) stays bf16: scores contract over only
  d=128 so DoubleRow cannot beat bf16 there, and fp8 P would need 2 extra
  full passes over the score matrix.
- Phases are interleaved per t-slice (projection slice sl, then attention
  over slice sl, then its output projection) with a static PSUM layout that
  fits all of it in the 8 banks: plin 2x[P,512], av [P,512], rs [P,512] and
  one 2-buffer [P,1024] ring shared by score tiles, the rope rotate-half
  matmul, the v-transpose (fp32) and the o-projection accumulators. The
  projection DoubleRow matmuls act as PE filler inside the exp-latency
  stalls of the attention pipeline (attention is high_priority).
- Softmax in S^T layout ([s,t], exp without max-subtraction; logits O(1)).
  Rowsums via DVE tree-adds of 4 probability segments + one ones-matmul per
  octet. Diagonal-region masking via skipped columns + memset (DVE) + tri
  multiply.
- fp16 output partials, one store per 128-row tile.
"""

import numpy as np
import ml_dtypes
from contextlib import ExitStack

from concourse import bacc, tile, mybir
from concourse.bass_utils import run_bass_kernel_spmd

NQ, NKV, D = 16, 2, 128
HID = 2048
T = 4096
SCALE = D ** -0.5
NCORES = 8
HPC = NQ // NCORES          # q heads per core
P = 128                     # partitions
TS = 512                    # t-slice width (matmul moving free dim)
NT = T // P                 # 32 t tiles
NSL = T // TS               # 8 t slices
HO = HID // P               # 16 hidden k-tiles
BF16 = mybir.dt.bfloat16
F32 = mybir.dt.float32
F16 = mybir.dt.float16
F8 = mybir.dt.float8e4
AF = mybir.ActivationFunctionType
ALU = mybir.AluOpType
DR = mybir.MatmulPerfMode.DoubleRow
NPBF16 = ml_dtypes.bfloat16
NPF8 = ml_dtypes.float8_e4m3

WSCL = 64.0                 # fp8 pre-scale on weights
AOSCL = 8.0                 # fp8 pre-scale on attention output (via ones/8)

_CACHE = {}


def _emit(nc, io, o_dram):
    with ExitStack() as top:
        tc = top.enter_context(tile.TileContext(nc))
        const = top.enter_context(tc.tile_pool(name="const", bufs=1))
        persist = top.enter_context(tc.tile_pool(name="persist", bufs=1))
        xs_pool = top.enter_context(tc.tile_pool(name="xs", bufs=2))
        rtmp = top.enter_context(tc.tile_pool(name="rtmp", bufs=3))
        ptp = top.enter_context(tc.tile_pool(name="ptp", bufs=11))
        nstage = top.enter_context(tc.tile_pool(name="nstage", bufs=2))
        qtmp = top.enter_context(tc.tile_pool(name="qtmp", bufs=5))
        ostage = top.enter_context(tc.tile_pool(name="ostage", bufs=3))
        # PSUM (16KB): plin 2KB + av 2KB + rs 2KB + opp 2KB + st ring 2x4KB.
        # opp gets its own bank so the o-projection never sits in the st
        # ring (that would serialize the next slice's scores behind it).
        ppsum = top.enter_context(tc.tile_pool(name="ppsum", bufs=1, space="PSUM"))
        avp = top.enter_context(tc.tile_pool(name="avp", bufs=1, space="PSUM"))
        rsp = top.enter_context(tc.tile_pool(name="rsp", bufs=1, space="PSUM"))
        opp = top.enter_context(tc.tile_pool(name="opp", bufs=1, space="PSUM"))
        stp = top.enter_context(tc.tile_pool(name="stp", bufs=2, space="PSUM"))

        def cload(name, shape, dt, eng=None):
            t = const.tile(shape, dt, tag=name)
            (eng or nc.sync).dma_start(t[:], io[name][:])
            return t

        # Load order: hi parts first (term 1 = w_hi*x_hi can start), the
        # small phase-A constants next, lo parts after, phase-B/C later.
        # HWDGE issues one DMA per ~650ns serially, so the startup uses few,
        # large DMAs and blobbed constants.
        wqs = const.tile([P, HO, 2, HPC * D], F8, tag="wqs")
        wqh, wql = wqs[:, :, 0, :], wqs[:, :, 1, :]
        xt0 = xs_pool.tile([P, HO, 2, TS], F8, tag="xt", name="xt0")
        # cb32 = [bq(2) | bk(1) | bv(1) | iden(128)]; cb16 = [rot|ones|tri]
        cb32 = const.tile([P, 4 + P], F32, tag="cb32")
        bq, bk, bv = cb32[:, 0:HPC], cb32[:, 2:3], cb32[:, 3:4]
        iden = cb32[:, 4:4 + P]
        nc.sync.dma_start(xt0[:, :4], io["xts"][0, :, :4])
        nc.sync.dma_start(wqs[:, :8], io["wqs"][:, :8])
        nc.sync.dma_start(xt0[:, 4:8], io["xts"][0, :, 4:8])
        nc.sync.dma_start(wqs[:, 8:], io["wqs"][:, 8:])
        nc.sync.dma_start(xt0[:, 8:12], io["xts"][0, :, 8:12])
        wks = cload("wks", [P, HO, 2, D], F8)
        wkh, wkl = wks[:, :, 0, :], wks[:, :, 1, :]
        nc.sync.dma_start(xt0[:, 12:], io["xts"][0, :, 12:])
        wvs = cload("wvs", [P, HO, 2, D], F8)
        wvh, wvl = wvs[:, :, 0, :], wvs[:, :, 1, :]
        nc.sync.dma_start(cb32[:], io["cb32"][:])
        cb16 = cload("cb16", [P, 3 * P], BF16)
        rot, ones, tri = cb16[:, :P], cb16[:, P:2 * P], cb16[:, 2 * P:]
        # cos/sin stream per t-slice as one [P, 2, TS] DMA; slice 0 right
        # after the x0 lo chunks (rope q0 needs it at ~12us)
        csT = const.tile([P, 2, T], BF16, tag="cst")
        cosT, sinT = csT[:, 0, :], csT[:, 1, :]
        tsl0 = slice(0, TS)
        nc.sync.dma_start(csT[:, :, tsl0], io["cst"][:, :, tsl0])
        # wo is needed by oproj(0) (~40us in); emitted before the loop for
        # write-then-read ordering but loaded inside slice 0 (see below) so
        # it doesn't displace the slice-0/1 criticals in the DMA queue
        woh = const.tile([P, HPC, HID], F8, tag="woth")
        wol = const.tile([P, HPC, HID], F8, tag="wotl")
        xnext = [None, None]

        qT = persist.tile([P, HPC, T], BF16, tag="qT")     # [d, h, t]
        kT = persist.tile([P, T], BF16, tag="kT")          # [d, s]
        vN = persist.tile([P, NT, P], BF16, tag="vN")      # [s_in, s_tile, d]
        aoh = persist.tile([P, HPC, T], F8, tag="aoh")     # [d, h, t] hi
        aol = persist.tile([P, HPC, T], F8, tag="aol")     # [d, h, t] lo

        x_t = {0: xt0}

        def load_x_cs(sl):
            tsl = slice(sl * TS, (sl + 1) * TS)
            nx = xs_pool.tile([P, HO, 2, TS], F8, tag="xt", name="xnx")
            nc.sync.dma_start(nx[:], io["xts"][sl])
            x_t[sl] = nx
            nc.sync.dma_start(csT[:, :, tsl], io["cst"][:, :, tsl])

        def emit_A_jobs(sl, jobs):
            # fp8-DR projections for the given jobs of slice sl; all PSUM
            # (pl accumulator, rope rot, v transpose) lives in the plin ring
            # so attention's st ring never waits on projection work. For the
            # first slices (attention still tiny, st ring idle) every other
            # job borrows the st ring to double-buffer the startup.
            tsl = slice(sl * TS, (sl + 1) * TS)
            xth = x_t[sl][:, :, 0, :]
            xtl = x_t[sl][:, :, 1, :]
            early = sl <= 4
            for jidx, (kind, h) in enumerate(jobs):
                if kind == "q":
                    wh_ap = wqh[:, :, h * D:(h + 1) * D]
                    wl_ap = wql[:, :, h * D:(h + 1) * D]
                    b_ap = bq[:, h:h + 1]
                elif kind == "k":
                    wh_ap, wl_ap, b_ap = wkh, wkl, bk
                else:
                    wh_ap, wl_ap, b_ap = wvh, wvl, bv
                if early and jidx % 2 == 1:
                    pl2 = stp.tile([P, 2 * TS], F32, tag="st", name="pl2")
                    pl = pl2[:, :TS]
                else:
                    pl = ppsum.tile([P, TS], F32, tag="plin", name="pl")[:]
                # hi-x terms first: x_lo arrives one prefetch step later
                terms = [(wh_ap, xth), (wl_ap, xth), (wh_ap, xtl)]
                nmm = len(terms) * (HO // 2)
                i = 0
                for w_ap, x_ap in terms:
                    for hp in range(HO // 2):
                        hs = slice(2 * hp, 2 * hp + 2)
                        nc.tensor.matmul(pl[:], w_ap[:, hs, :], x_ap[:, hs, :],
                                         start=(i == 0), stop=(i == nmm - 1),
                                         perf_mode=DR)
                        i += 1
                if kind in ("q", "k"):
                    # lin = pl/64 + b (bf16); tcos = lin*cos on Pool;
                    # rp = rot@lin on PE; tsin = rp*sin on DVE
                    lin = rtmp.tile([P, TS], BF16, tag="lin")
                    nc.vector.tensor_scalar(lin[:], pl[:], 1.0 / WSCL, b_ap,
                                            ALU.mult, ALU.add)
                    tcos = rtmp.tile([P, TS], F32, tag="tcos")
                    nc.gpsimd.tensor_mul(tcos[:], lin[:], cosT[:, tsl])
                    if early:
                        rp2 = stp.tile([P, 2 * TS], F32, tag="st", name="rp2")
                        rp = rp2[:, :TS]
                    else:
                        rp = ppsum.tile([P, TS], F32, tag="plin", name="rp")[:]
                    nc.tensor.matmul(rp[:], rot[:], lin[:], start=True,
                                     stop=True)
                    tsin = rtmp.tile([P, TS], F32, tag="tsin")
                    nc.vector.tensor_mul(tsin[:], rp[:], sinT[:, tsl])
                    dst = qT[:, h, tsl] if kind == "q" else kT[:, tsl]
                    nc.vector.tensor_add(dst, tsin[:], tcos[:])
                else:
                    # v: lin in fp32 so the PE transpose (fp32, 2 cyc/row)
                    # can land in the fp32 plin ring
                    linv = rtmp.tile([P, TS], F32, tag="linv", bufs=2)
                    nc.vector.tensor_scalar(linv[:], pl[:], 1.0 / WSCL, b_ap,
                                            ALU.mult, ALU.add)
                    vt = ppsum.tile([P, TS], F32, tag="plin", name="vt")
                    for tt in range(TS // P):
                        nc.tensor.transpose(vt[:, tt * P:(tt + 1) * P],
                                            linv[:, tt * P:(tt + 1) * P],
                                            iden[:])
                    nc.vector.tensor_copy(vN[:, 4 * sl:4 * (sl + 1), :], vt[:])

        def emit_B_head(sl, h):
            # attention for head h over t-slice sl, S^T-layout flash
            tsl = slice(sl * TS, (sl + 1) * TS)
            n_s = 4 * sl + 4          # causal s tiles for this slice
            ng = n_s // 2
            # high priority: when both are ready the PE prefers attention;
            # projection work drifts into the exp stall windows as filler
            with tc.high_priority(offset=110):
                av = avp.tile([P, TS], F32, tag="av")
                rs = rsp.tile([P, TS], F32, tag="rs")
                pend3 = None
                pend4 = None
                rcnt = 0
                t4cnt = 0
                n_t4 = (sl + 2) // 2       # octet-level fold items
                n_rsmm = (n_t4 + 1) // 2   # one PE matmul per 16 segments
                pt_even = None
                for g in range(ng):
                    st = stp.tile([P, 2 * TS], F32, tag="st")
                    pt = ptp.tile([P, 2 * TS], BF16, tag="pt")
                    # off > 0 marks a diagonal-region s tile: its first
                    # off t-columns are fully masked -> skip them in the
                    # matmuls and exp, mask only the diagonal block.
                    offs = [max(2 * g + i - 4 * sl, 0) * P for i in range(2)]
                    for i in range(2):
                        s_tile = 2 * g + i
                        off = offs[i]
                        nc.tensor.matmul(
                            st[:, i * TS + off:(i + 1) * TS],
                            kT[:, s_tile * P:(s_tile + 1) * P],
                            qT[:, h, sl * TS + off:(sl + 1) * TS],
                            start=True, stop=True)
                    if offs[1] == 0:
                        nc.scalar.activation(pt[:], st[:], AF.Exp, scale=SCALE)
                    else:
                        # one exp spanning both segments (incl. the stale
                        # gap [TS : TS+off1], zeroed right after) -- saves
                        # the ACT call overhead of a second exp
                        off0, off1 = offs
                        nc.scalar.activation(pt[:, off0:], st[:, off0:],
                                             AF.Exp, scale=SCALE)
                        for i in range(2):
                            c0 = i * TS + offs[i]
                            nc.vector.tensor_mul(pt[:, c0:c0 + P],
                                                 pt[:, c0:c0 + P], tri[:])
                        # the zeroed regions feed only the rowsum folds (the
                        # PV segments skip them), so they're off the PV path
                        if off0:
                            nc.vector.memset(pt[:, :off0], 0.0)
                        nc.vector.memset(pt[:, TS:TS + off1], 0.0)
                    for i in range(2):
                        s_tile = 2 * g + i
                        off = offs[i]
                        seg = pt[:, i * TS + off:(i + 1) * TS]
                        nc.tensor.matmul(av[:, off:TS], vN[:, s_tile, :], seg,
                                         start=(s_tile == 0),
                                         stop=(s_tile == n_s - 1))
                    # Rowsum: diag segments are zero-padded below the
                    # diagonal, so full-width tree adds on the DVE fold 4
                    # segments into one tile -> one PE matmul per octet.
                    if g % 2 == 0:
                        pt_even = pt
                    else:
                        qd = g // 2
                        t1 = qtmp.tile([P, TS], BF16, tag="q1")
                        nc.gpsimd.tensor_add(t1[:], pt_even[:, :TS],
                                             pt_even[:, TS:])
                        t2 = qtmp.tile([P, TS], BF16, tag="q2")
                        nc.vector.tensor_add(t2[:], pt[:, :TS], pt[:, TS:])
                        t3 = qtmp.tile([P, TS], BF16, tag="q3",
                                       name=f"t3{qd % 2}")
                        nc.vector.tensor_add(t3[:], t1[:], t2[:])
                        if pend3 is None and qd < sl:
                            pend3 = t3      # wait for a partner quad
                        else:
                            if pend3 is not None:
                                t4 = qtmp.tile([P, TS], BF16, tag="q4",
                                               name=f"t4{t4cnt % 2}")
                                nc.vector.tensor_add(t4[:], pend3[:], t3[:])
                                pend3 = None
                            else:
                                t4 = t3     # odd leftover quad
                            # pair octets too: one matmul per 16 segments
                            if pend4 is None and t4cnt + 1 < n_t4:
                                pend4 = t4
                            else:
                                if pend4 is not None:
                                    t5 = qtmp.tile([P, TS], BF16, tag="q4",
                                                   name="t5")
                                    nc.vector.tensor_add(t5[:], pend4[:],
                                                         t4[:])
                                    rhs16 = t5
                                    pend4 = None
                                else:
                                    rhs16 = t4  # odd leftover octet
                                nc.tensor.matmul(rs[:], ones[:], rhs16[:],
                                                 start=(rcnt == 0),
                                                 stop=(rcnt == n_rsmm - 1))
                                rcnt += 1
                            t4cnt += 1
                # rec = 8/rowsum (ones is 1/8) -> aom = 8*attn_out; only one
                # PSUM operand allowed per DVE op, so rec stages through SBUF.
                # fp8 hi/lo split on the Pool engine (SBUF-only reads).
                # The last slice drains in 128-col stages on the DVE so the
                # o-projection can chase the normalize down the tail.
                rec = nstage.tile([P, TS], F32, tag="rec")
                aom = nstage.tile([P, TS], F32, tag="aom")
                if sl == NSL - 1:
                    for q4 in range(4):
                        cq = slice(q4 * P, (q4 + 1) * P)
                        tq = slice(sl * TS + q4 * P, sl * TS + (q4 + 1) * P)
                        nc.vector.reciprocal(rec[:, cq], rs[:, cq])
                        nc.vector.tensor_mul(aom[:, cq], av[:, cq], rec[:, cq])
                        nc.vector.tensor_copy(aoh[:, h, tq], aom[:, cq])
                        nc.vector.tensor_tensor(aol[:, h, tq], aom[:, cq],
                                                aoh[:, h, tq], ALU.subtract)
                else:
                    nc.vector.reciprocal(rec[:], rs[:])
                    nc.vector.tensor_mul(aom[:], av[:], rec[:])
                    nc.gpsimd.tensor_copy(aoh[:, h, tsl], aom[:])
                    nc.gpsimd.tensor_tensor(aol[:, h, tsl], aom[:],
                                            aoh[:, h, tsl], ALU.subtract)

        def emit_C(sl, tiles=range(4)):
            # output projection, fp8-DR over (d x head) pairs, 3 cross terms.
            # Normally one 512-col accumulator at a time in the private opp
            # bank (copy-paced, pure filler alongside the next slice's
            # attention); the last slice borrows the st ring (attention is
            # done) to double-buffer the drain.
            fin = sl == NSL - 1
            for tt4 in tiles:
                t_tile = 4 * sl + tt4
                trow = slice(t_tile * P, (t_tile + 1) * P)
                ot = ostage.tile([P, HID], F16, tag="ot")
                for upair in range(2):
                    if fin:
                        # attention is finished: rotate the o-proj
                        # accumulators over the st ring AND the freed av/rs
                        # banks so the DR bursts fully overlap the copies
                        if upair == 0:
                            op2 = stp.tile([P, 2 * TS], F32, tag="st",
                                           name="op2")
                            ops = [op2[:, :TS], op2[:, TS:]]
                        else:
                            opa = avp.tile([P, TS], F32, tag="av", name="opa")
                            opb = rsp.tile([P, TS], F32, tag="rs", name="opb")
                            ops = [opa[:], opb[:]]
                    else:
                        ops = None
                    for ui in range(2):
                        u0 = (upair * 2 + ui) * TS
                        usl = slice(u0, u0 + TS)
                        if fin:
                            op = ops[ui]
                        else:
                            op = opp.tile([P, TS], F32, tag="op",
                                          name="op")[:]
                        nc.tensor.matmul(op, aoh[:, :, trow],
                                         woh[:, :, usl],
                                         start=True, stop=False, perf_mode=DR)
                        nc.tensor.matmul(op, aoh[:, :, trow],
                                         wol[:, :, usl],
                                         start=False, stop=False, perf_mode=DR)
                        nc.tensor.matmul(op, aol[:, :, trow],
                                         woh[:, :, usl],
                                         start=False, stop=True, perf_mode=DR)
                        if ui == 0:
                            nc.vector.tensor_scalar(
                                ot[:, usl], op, 1.0 / (WSCL * AOSCL),
                                None, ALU.mult)
                        else:
                            nc.scalar.activation(
                                ot[:, usl], op, AF.Copy,
                                scale=1.0 / (WSCL * AOSCL))
                    if fin:
                        # finer store granularity so the kernel-exit drain
                        # waits on a small final DMA
                        uhalf = slice(upair * 2 * TS, (upair + 1) * 2 * TS)
                        nc.sync.dma_start(o_dram[trow, uhalf], ot[:, uhalf])
                if not fin:
                    nc.sync.dma_start(o_dram[trow, :], ot[:])

        # Emission order: A(sl+1)'s jobs are emitted interleaved between
        # B(sl)'s heads so the list scheduler sees projection filler near
        # the attention stalls it should fill. cos/sin(0) and x(0) were
        # loaded by the chunked startup loads above.
        emit_A_jobs(0, [("q", 0), ("q", 1), ("k", 0), ("v", 0)])
        for sl in range(NSL):
            if sl + 1 < NSL:
                load_x_cs(sl + 1)
            if sl == 0:
                nc.sync.dma_start(woh[:], io["woth"][:])
                nc.sync.dma_start(wol[:], io["wotl"][:])
            emit_B_head(sl, 0)
            if sl + 1 < NSL:
                # q0 and k first: B(sl+1) head 0 needs them; q1 can finish
                # during B(sl+1, 0)
                emit_A_jobs(sl + 1, [("q", 0), ("k", 0)])
            elif sl == NSL - 1:
                # deferred C(NSL-2): the only PE filler left for the last
                # slice's exp-paced attention (A is done by now)
                emit_C(NSL - 2)
            emit_B_head(sl, 1)
            if sl + 1 < NSL:
                emit_A_jobs(sl + 1, [("v", 0), ("q", 1)])
            if sl < NSL - 2 or sl == NSL - 1:
                emit_C(sl)


def _build_nc():
    nc = bacc.Bacc("TRN2", target_bir_lowering=False, debug=False,
                   enable_asserts=False, num_devices=NCORES)
    io = {}

    def din(name, shape, dt):
        io[name] = nc.dram_tensor(name, shape, dt, kind="ExternalInput").ap()

    din("xts", [NSL, P, HO, 2, TS], F8)      # x^T hi|lo interleaved
    din("wqs", [P, HO, 2, HPC * D], F8)      # 64*wq hi|lo interleaved
    din("wks", [P, HO, 2, D], F8)
    din("wvs", [P, HO, 2, D], F8)
    din("woth", [P, HPC, HID], F8)
    din("wotl", [P, HPC, HID], F8)
    din("cst", [P, 2, T], BF16)              # [cos; sin], transposed
    din("cb32", [P, 4 + P], F32)             # [bq | bk | bv | iden]
    din("cb16", [P, 3 * P], BF16)            # [rot | ones/8 | tri]
    o = nc.dram_tensor("o_part", [T, HID], F16, kind="ExternalOutput").ap()
    _emit(nc, io, o)
    nc.compile()
    return nc


def _get_nc():
    if "nc" not in _CACHE:
        _CACHE["nc"] = _build_nc()
    return _CACHE["nc"]


def _split8(a):
    hi = a.astype(NPF8)
    lo = (a - hi.astype(np.float32)).astype(NPF8)
    return hi, lo


def _consts():
    """cb16 blob [P, 3P] = [rot | ones/8 | tri] in bf16."""
    if "consts" in _CACHE:
        return _CACHE["consts"]
    # rotate_half as a matmul on lin: out[d,t] = sum_e R[e,d] lin[e,t]
    R = np.zeros((P, P), np.float32)
    for e in range(P // 2):
        R[e, e + P // 2] = 1.0      # d >= 64 takes +q[d-64]
    for e in range(P // 2, P):
        R[e, e - P // 2] = -1.0     # d < 64 takes -q[d+64]
    onesm = np.full((P, P), 1.0 / AOSCL, np.float32)
    tri = np.triu(np.ones((P, P), np.float32))
    _CACHE["consts"] = np.ascontiguousarray(
        np.concatenate([R, onesm, tri], axis=1)).astype(NPBF16)
    return _CACHE["consts"]


def kernel(x, cos, sin, wq, bq, wk, bk, wv, bv, wo):
    x = np.asarray(x, dtype=np.float32)
    cos = np.asarray(cos, dtype=np.float32)
    sin = np.asarray(sin, dtype=np.float32)
    wq = np.asarray(wq, dtype=np.float32)
    bq = np.asarray(bq, dtype=np.float32)
    wk = np.asarray(wk, dtype=np.float32)
    bk = np.asarray(bk, dtype=np.float32)
    wv = np.asarray(wv, dtype=np.float32)
    bv = np.asarray(bv, dtype=np.float32)
    wo = np.asarray(wo, dtype=np.float32)

    nc = _get_nc()
    cb16 = _consts()

    # x^T tiled: xtt[sl, p, ho, c] = x[0, sl*TS + c, ho*P + p]; fp8 hi/lo
    xT = np.ascontiguousarray(x[0].T)                             # [HID, T]
    xtt = np.ascontiguousarray(
        xT.reshape(HO, P, NSL, TS).transpose(2, 1, 0, 3))         # [NSL,P,HO,TS]
    xtth, xttl = _split8(xtt)
    xts = np.ascontiguousarray(
        np.stack([xtth, xttl], axis=3))                   # [NSL,P,HO,2,TS]

    cst = np.ascontiguousarray(
        np.stack([cos.T, sin.T], axis=1)).astype(NPBF16)          # [P, 2, T]

    def wtile(wslice):  # [J, HID] -> [P, HO, J] with h = ho*P + p
        J = wslice.shape[0]
        return np.ascontiguousarray(
            wslice.T.reshape(HO, P, J).transpose(1, 0, 2)) * WSCL

    in_maps = []
    for c in range(NCORES):
        j0 = c * HPC * D
        kvh = c // (NCORES // NKV)
        wqs = np.ascontiguousarray(np.stack(
            _split8(wtile(wq[j0:j0 + HPC * D])), axis=2))     # [P, HO, 2, 256]
        wks = np.ascontiguousarray(np.stack(
            _split8(wtile(wk[kvh * D:(kvh + 1) * D])), axis=2))
        wvs = np.ascontiguousarray(np.stack(
            _split8(wtile(wv[kvh * D:(kvh + 1) * D])), axis=2))
        # woT: [P, HPC, HID] with j = h*P + p
        wot = np.ascontiguousarray(
            wo[:, j0:j0 + HPC * D].T.reshape(HPC, P, HID)
            .transpose(1, 0, 2)) * WSCL
        woth, wotl = _split8(wot)
        bqt = np.ascontiguousarray(bq[j0:j0 + HPC * D].reshape(HPC, P).T)
        bkt = bk[kvh * D:(kvh + 1) * D].reshape(P, 1)
        bvt = bv[kvh * D:(kvh + 1) * D].reshape(P, 1)
        cb32 = np.concatenate(
            [bqt, bkt, bvt, np.eye(P)], axis=1).astype(np.float32)
        in_maps.append({
            "xts": xts,
            "wqs": wqs, "wks": wks, "wvs": wvs,
            "woth": woth, "wotl": wotl,
            "cst": cst, "cb32": cb32, "cb16": cb16,
        })

    res = run_bass_kernel_spmd(nc, in_maps, list(range(NCORES)))
    out = np.zeros((T, HID), np.float32)
    for c in range(NCORES):
        out += res.results[c]["o_part"].astype(np.float32)
    return out.reshape(1, T, HID)


# revision 78
# speedup vs baseline: 1.0045x; 1.0045x over previous
"""Trainium2 Bass kernel for GQA causal attention (nn_Attention_89816356094768).

Math (per reference):
  q = x @ wq.T + bq ; k = x @ wk.T + bk ; v = x @ wv.T + bv
  RoPE on q, k; S = q @ k.T * D**-0.5 with causal mask; P = softmax(S)
  out = (P @ v) reassembled over heads @ wo.T

Sharding: tensor-parallel over heads across 8 cores. Core c owns q heads
(2c, 2c+1) and kv head c//4. Each core computes its two heads' attention and
a row-parallel partial of the output projection; the host sums the 8 partials
(fp16 partials, summed in fp32).

Perf structure (v4):
- q/k/v and output projections run as fp8e4m3 DoubleRow matmuls (contraction
  pairs packed on the partition dim, 4x bf16 throughput per k-tile in the
  cost model). Precision is kept with a hi+lo split of both operands and 3
  cross terms (hi*hi + hi*lo + lo*hi) = 0.75x the bf16 PE cycles. Weights are
  pre-scaled by 64 on the host (w std 0.012 is below e4m3's normal range);
  the 1/64 is folded into the lin copy / rope products. The attention output
  is scaled by 8 via the rowsum ones matrix (host-side ones/8) before its
  fp8 hi/lo split on the Pool engine, and the o-projection PSUM is divided
  by 512 in the PSUM->fp16 copies (split DVE/ACT).
- Attention (scores/softmax/PV/rowsum) stays bf16: scores contract over only
  d=128 so DoubleRow cannot beat bf16 there, and fp8 P would need 2 extra
  full passes over the score matrix.
- Phases are interleaved per t-slice (projection slice sl, then attention
  over slice sl, then its output projection) with a static PSUM layout that
  fits all of it in the 8 banks: plin 2x[P,512], av [P,512], rs [P,512] and
  one 2-buffer [P,1024] ring shared by score tiles, the rope rotate-half
  matmul, the v-transpose (fp32) and the o-projection accumulators. The
  projection DoubleRow matmuls act as PE filler inside the exp-latency
  stalls of the attention pipeline (attention is high_priority).
- Softmax in S^T layout ([s,t], exp without max-subtraction; logits O(1)).
  Rowsums via DVE tree-adds of 4 probability segments + one ones-matmul per
  octet. Diagonal-region masking via skipped columns + memset (DVE) + tri
  multiply.
- fp16 output partials, one store per 128-row tile.
"""

import numpy as np
import ml_dtypes
from contextlib import ExitStack

from concourse import bacc, tile, mybir
from concourse.bass_utils import run_bass_kernel_spmd

NQ, NKV, D = 16, 2, 128
HID = 2048
T = 4096
SCALE = D ** -0.5
NCORES = 8
HPC = NQ // NCORES          # q heads per core
P = 128                     # partitions
TS = 512                    # t-slice width (matmul moving free dim)
NT = T // P                 # 32 t tiles
NSL = T // TS               # 8 t slices
HO = HID // P               # 16 hidden k-tiles
BF16 = mybir.dt.bfloat16
F32 = mybir.dt.float32
F16 = mybir.dt.float16
F8 = mybir.dt.float8e4
AF = mybir.ActivationFunctionType
ALU = mybir.AluOpType
DR = mybir.MatmulPerfMode.DoubleRow
NPBF16 = ml_dtypes.bfloat16
NPF8 = ml_dtypes.float8_e4m3

WSCL = 64.0                 # fp8 pre-scale on weights
AOSCL = 8.0                 # fp8 pre-scale on attention output (via ones/8)

_CACHE = {}


def _emit(nc, io, o_dram):
    with ExitStack() as top:
        tc = top.enter_context(tile.TileContext(nc))
        const = top.enter_context(tc.tile_pool(name="const", bufs=1))
        persist = top.enter_context(tc.tile_pool(name="persist", bufs=1))
        xs_pool = top.enter_context(tc.tile_pool(name="xs", bufs=2))
        rtmp = top.enter_context(tc.tile_pool(name="rtmp", bufs=3))
        ptp = top.enter_context(tc.tile_pool(name="ptp", bufs=11))
        nstage = top.enter_context(tc.tile_pool(name="nstage", bufs=2))
        qtmp = top.enter_context(tc.tile_pool(name="qtmp", bufs=5))
        ostage = top.enter_context(tc.tile_pool(name="ostage", bufs=3))
        # PSUM (16KB): plin 2KB + av 2KB + rs 2KB + opp 2KB + st ring 2x4KB.
        # opp gets its own bank so the o-projection never sits in the st
        # ring (that would serialize the next slice's scores behind it).
        ppsum = top.enter_context(tc.tile_pool(name="ppsum", bufs=1, space="PSUM"))
        avp = top.enter_context(tc.tile_pool(name="avp", bufs=1, space="PSUM"))
        rsp = top.enter_context(tc.tile_pool(name="rsp", bufs=1, space="PSUM"))
        opp = top.enter_context(tc.tile_pool(name="opp", bufs=1, space="PSUM"))
        stp = top.enter_context(tc.tile_pool(name="stp", bufs=2, space="PSUM"))

        def cload(name, shape, dt, eng=None):
            t = const.tile(shape, dt, tag=name)
            (eng or nc.sync).dma_start(t[:], io[name][:])
            return t

        # Load order: hi parts first (term 1 = w_hi*x_hi can start), the
        # small phase-A constants next, lo parts after, phase-B/C later.
        # HWDGE issues one DMA per ~650ns serially, so the startup uses few,
        # large DMAs and blobbed constants.
        wqh = const.tile([P, HO, HPC * D], F8, tag="wqth")
        wql = const.tile([P, HO, HPC * D], F8, tag="wqtl")
        xt0 = xs_pool.tile([P, HO, 2, TS], F8, tag="xt", name="xt0")
        # cb32 = [bq(2) | bk(1) | bv(1) | iden(128)]; cb16 = [rot|ones|tri]
        cb32 = const.tile([P, 4 + P], F32, tag="cb32")
        bq, bk, bv = cb32[:, 0:HPC], cb32[:, 2:3], cb32[:, 3:4]
        iden = cb32[:, 4:4 + P]
        nc.sync.dma_start(xt0[:, :4], io["xts"][0, :, :4])
        nc.sync.dma_start(wqh[:, :8, :], io["wqth"][:, :8, :])
        nc.sync.dma_start(xt0[:, 4:8], io["xts"][0, :, 4:8])
        nc.sync.dma_start(wqh[:, 8:, :], io["wqth"][:, 8:, :])
        nc.sync.dma_start(xt0[:, 8:12], io["xts"][0, :, 8:12])
        nc.sync.dma_start(wql[:], io["wqtl"][:])
        nc.sync.dma_start(xt0[:, 12:], io["xts"][0, :, 12:])
        nc.sync.dma_start(cb32[:], io["cb32"][:])
        wkh = cload("wkth", [P, HO, D], F8)
        wvh = cload("wvth", [P, HO, D], F8)
        cb16 = cload("cb16", [P, 3 * P], BF16)
        rot, ones, tri = cb16[:, :P], cb16[:, P:2 * P], cb16[:, 2 * P:]
        # cos/sin stream per t-slice as one [P, 2, TS] DMA; slice 0 right
        # after the x0 lo chunks (rope q0 needs it at ~12us)
        csT = const.tile([P, 2, T], BF16, tag="cst")
        cosT, sinT = csT[:, 0, :], csT[:, 1, :]
        tsl0 = slice(0, TS)
        nc.sync.dma_start(csT[:, :, tsl0], io["cst"][:, :, tsl0])
        wkl = cload("wktl", [P, HO, D], F8)
        wvl = cload("wvtl", [P, HO, D], F8)
        # wo is needed by oproj(0) (~40us in); emitted before the loop for
        # write-then-read ordering but loaded inside slice 0 (see below) so
        # it doesn't displace the slice-0/1 criticals in the DMA queue
        woh = const.tile([P, HPC, HID], F8, tag="woth")
        wol = const.tile([P, HPC, HID], F8, tag="wotl")
        xnext = [None, None]

        qT = persist.tile([P, HPC, T], BF16, tag="qT")     # [d, h, t]
        kT = persist.tile([P, T], BF16, tag="kT")          # [d, s]
        vN = persist.tile([P, NT, P], BF16, tag="vN")      # [s_in, s_tile, d]
        aoh = persist.tile([P, HPC, T], F8, tag="aoh")     # [d, h, t] hi
        aol = persist.tile([P, HPC, T], F8, tag="aol")     # [d, h, t] lo

        x_t = {0: xt0}

        def load_x_cs(sl):
            tsl = slice(sl * TS, (sl + 1) * TS)
            nx = xs_pool.tile([P, HO, 2, TS], F8, tag="xt", name="xnx")
            nc.sync.dma_start(nx[:], io["xts"][sl])
            x_t[sl] = nx
            nc.sync.dma_start(csT[:, :, tsl], io["cst"][:, :, tsl])

        def emit_A_jobs(sl, jobs):
            # fp8-DR projections for the given jobs of slice sl; all PSUM
            # (pl accumulator, rope rot, v transpose) lives in the plin ring
            # so attention's st ring never waits on projection work. For the
            # first slices (attention still tiny, st ring idle) every other
            # job borrows the st ring to double-buffer the startup.
            tsl = slice(sl * TS, (sl + 1) * TS)
            xth = x_t[sl][:, :, 0, :]
            xtl = x_t[sl][:, :, 1, :]
            early = sl <= 4
            for jidx, (kind, h) in enumerate(jobs):
                if kind == "q":
                    wh_ap = wqh[:, :, h * D:(h + 1) * D]
                    wl_ap = wql[:, :, h * D:(h + 1) * D]
                    b_ap = bq[:, h:h + 1]
                elif kind == "k":
                    wh_ap, wl_ap, b_ap = wkh, wkl, bk
                else:
                    wh_ap, wl_ap, b_ap = wvh, wvl, bv
                if early and jidx % 2 == 1:
                    pl2 = stp.tile([P, 2 * TS], F32, tag="st", name="pl2")
                    pl = pl2[:, :TS]
                else:
                    pl = ppsum.tile([P, TS], F32, tag="plin", name="pl")[:]
                # hi-x terms first: x_lo arrives one prefetch step later
                terms = [(wh_ap, xth), (wl_ap, xth), (wh_ap, xtl)]
                nmm = len(terms) * (HO // 2)
                i = 0
                for w_ap, x_ap in terms:
                    for hp in range(HO // 2):
                        hs = slice(2 * hp, 2 * hp + 2)
                        nc.tensor.matmul(pl[:], w_ap[:, hs, :], x_ap[:, hs, :],
                                         start=(i == 0), stop=(i == nmm - 1),
                                         perf_mode=DR)
                        i += 1
                if kind in ("q", "k"):
                    # lin = pl/64 + b (bf16); tcos = lin*cos on Pool;
                    # rp = rot@lin on PE; tsin = rp*sin on DVE
                    lin = rtmp.tile([P, TS], BF16, tag="lin")
                    nc.vector.tensor_scalar(lin[:], pl[:], 1.0 / WSCL, b_ap,
                                            ALU.mult, ALU.add)
                    tcos = rtmp.tile([P, TS], F32, tag="tcos")
                    nc.gpsimd.tensor_mul(tcos[:], lin[:], cosT[:, tsl])
                    if early:
                        rp2 = stp.tile([P, 2 * TS], F32, tag="st", name="rp2")
                        rp = rp2[:, :TS]
                    else:
                        rp = ppsum.tile([P, TS], F32, tag="plin", name="rp")[:]
                    nc.tensor.matmul(rp[:], rot[:], lin[:], start=True,
                                     stop=True)
                    tsin = rtmp.tile([P, TS], F32, tag="tsin")
                    nc.vector.tensor_mul(tsin[:], rp[:], sinT[:, tsl])
                    dst = qT[:, h, tsl] if kind == "q" else kT[:, tsl]
                    nc.vector.tensor_add(dst, tsin[:], tcos[:])
                else:
                    # v: lin in fp32 so the PE transpose (fp32, 2 cyc/row)
                    # can land in the fp32 plin ring
                    linv = rtmp.tile([P, TS], F32, tag="linv", bufs=2)
                    nc.vector.tensor_scalar(linv[:], pl[:], 1.0 / WSCL, b_ap,
                                            ALU.mult, ALU.add)
                    vt = ppsum.tile([P, TS], F32, tag="plin", name="vt")
                    for tt in range(TS // P):
                        nc.tensor.transpose(vt[:, tt * P:(tt + 1) * P],
                                            linv[:, tt * P:(tt + 1) * P],
                                            iden[:])
                    nc.vector.tensor_copy(vN[:, 4 * sl:4 * (sl + 1), :], vt[:])

        def emit_B_head(sl, h):
            # attention for head h over t-slice sl, S^T-layout flash
            tsl = slice(sl * TS, (sl + 1) * TS)
            n_s = 4 * sl + 4          # causal s tiles for this slice
            ng = n_s // 2
            # high priority: when both are ready the PE prefers attention;
            # projection work drifts into the exp stall windows as filler
            with tc.high_priority(offset=110):
                av = avp.tile([P, TS], F32, tag="av")
                rs = rsp.tile([P, TS], F32, tag="rs")
                pend3 = None
                pend4 = None
                rcnt = 0
                t4cnt = 0
                n_t4 = (sl + 2) // 2       # octet-level fold items
                n_rsmm = (n_t4 + 1) // 2   # one PE matmul per 16 segments
                pt_even = None
                for g in range(ng):
                    st = stp.tile([P, 2 * TS], F32, tag="st")
                    pt = ptp.tile([P, 2 * TS], BF16, tag="pt")
                    # off > 0 marks a diagonal-region s tile: its first
                    # off t-columns are fully masked -> skip them in the
                    # matmuls and exp, mask only the diagonal block.
                    offs = [max(2 * g + i - 4 * sl, 0) * P for i in range(2)]
                    for i in range(2):
                        s_tile = 2 * g + i
                        off = offs[i]
                        nc.tensor.matmul(
                            st[:, i * TS + off:(i + 1) * TS],
                            kT[:, s_tile * P:(s_tile + 1) * P],
                            qT[:, h, sl * TS + off:(sl + 1) * TS],
                            start=True, stop=True)
                    if offs[1] == 0:
                        nc.scalar.activation(pt[:], st[:], AF.Exp, scale=SCALE)
                    else:
                        # one exp spanning both segments (incl. the stale
                        # gap [TS : TS+off1], zeroed right after) -- saves
                        # the ACT call overhead of a second exp
                        off0, off1 = offs
                        nc.scalar.activation(pt[:, off0:], st[:, off0:],
                                             AF.Exp, scale=SCALE)
                        for i in range(2):
                            c0 = i * TS + offs[i]
                            nc.vector.tensor_mul(pt[:, c0:c0 + P],
                                                 pt[:, c0:c0 + P], tri[:])
                        # the zeroed regions feed only the rowsum folds (the
                        # PV segments skip them), so they're off the PV path
                        if off0:
                            nc.vector.memset(pt[:, :off0], 0.0)
                        nc.vector.memset(pt[:, TS:TS + off1], 0.0)
                    for i in range(2):
                        s_tile = 2 * g + i
                        off = offs[i]
                        seg = pt[:, i * TS + off:(i + 1) * TS]
                        nc.tensor.matmul(av[:, off:TS], vN[:, s_tile, :], seg,
                                         start=(s_tile == 0),
                                         stop=(s_tile == n_s - 1))
                    # Rowsum: diag segments are zero-padded below the
                    # diagonal, so full-width tree adds on the DVE fold 4
                    # segments into one tile -> one PE matmul per octet.
                    if g % 2 == 0:
                        pt_even = pt
                    else:
                        qd = g // 2
                        t1 = qtmp.tile([P, TS], BF16, tag="q1")
                        nc.gpsimd.tensor_add(t1[:], pt_even[:, :TS],
                                             pt_even[:, TS:])
                        t2 = qtmp.tile([P, TS], BF16, tag="q2")
                        nc.vector.tensor_add(t2[:], pt[:, :TS], pt[:, TS:])
                        t3 = qtmp.tile([P, TS], BF16, tag="q3",
                                       name=f"t3{qd % 2}")
                        nc.vector.tensor_add(t3[:], t1[:], t2[:])
                        if pend3 is None and qd < sl:
                            pend3 = t3      # wait for a partner quad
                        else:
                            if pend3 is not None:
                                t4 = qtmp.tile([P, TS], BF16, tag="q4",
                                               name=f"t4{t4cnt % 2}")
                                nc.vector.tensor_add(t4[:], pend3[:], t3[:])
                                pend3 = None
                            else:
                                t4 = t3     # odd leftover quad
                            # pair octets too: one matmul per 16 segments
                            if pend4 is None and t4cnt + 1 < n_t4:
                                pend4 = t4
                            else:
                                if pend4 is not None:
                                    t5 = qtmp.tile([P, TS], BF16, tag="q4",
                                                   name="t5")
                                    nc.vector.tensor_add(t5[:], pend4[:],
                                                         t4[:])
                                    rhs16 = t5
                                    pend4 = None
                                else:
                                    rhs16 = t4  # odd leftover octet
                                nc.tensor.matmul(rs[:], ones[:], rhs16[:],
                                                 start=(rcnt == 0),
                                                 stop=(rcnt == n_rsmm - 1))
                                rcnt += 1
                            t4cnt += 1
                # rec = 8/rowsum (ones is 1/8) -> aom = 8*attn_out; only one
                # PSUM operand allowed per DVE op, so rec stages through SBUF.
                # fp8 hi/lo split on the Pool engine (SBUF-only reads).
                # The last slice drains in 128-col stages on the DVE so the
                # o-projection can chase the normalize down the tail.
                rec = nstage.tile([P, TS], F32, tag="rec")
                aom = nstage.tile([P, TS], F32, tag="aom")
                if sl == NSL - 1:
                    for q4 in range(4):
                        cq = slice(q4 * P, (q4 + 1) * P)
                        tq = slice(sl * TS + q4 * P, sl * TS + (q4 + 1) * P)
                        nc.vector.reciprocal(rec[:, cq], rs[:, cq])
                        nc.vector.tensor_mul(aom[:, cq], av[:, cq], rec[:, cq])
                        nc.vector.tensor_copy(aoh[:, h, tq], aom[:, cq])
                        nc.vector.tensor_tensor(aol[:, h, tq], aom[:, cq],
                                                aoh[:, h, tq], ALU.subtract)
                else:
                    nc.vector.reciprocal(rec[:], rs[:])
                    nc.vector.tensor_mul(aom[:], av[:], rec[:])
                    nc.gpsimd.tensor_copy(aoh[:, h, tsl], aom[:])
                    nc.gpsimd.tensor_tensor(aol[:, h, tsl], aom[:],
                                            aoh[:, h, tsl], ALU.subtract)

        def emit_C(sl, tiles=range(4)):
            # output projection, fp8-DR over (d x head) pairs, 3 cross terms.
            # Normally one 512-col accumulator at a time in the private opp
            # bank (copy-paced, pure filler alongside the next slice's
            # attention); the last slice borrows the st ring (attention is
            # done) to double-buffer the drain.
            fin = sl == NSL - 1
            for tt4 in tiles:
                t_tile = 4 * sl + tt4
                trow = slice(t_tile * P, (t_tile + 1) * P)
                ot = ostage.tile([P, HID], F16, tag="ot")
                for upair in range(2):
                    if fin:
                        # attention is finished: rotate the o-proj
                        # accumulators over the st ring AND the freed av/rs
                        # banks so the DR bursts fully overlap the copies
                        if upair == 0:
                            op2 = stp.tile([P, 2 * TS], F32, tag="st",
                                           name="op2")
                            ops = [op2[:, :TS], op2[:, TS:]]
                        else:
                            opa = avp.tile([P, TS], F32, tag="av", name="opa")
                            opb = rsp.tile([P, TS], F32, tag="rs", name="opb")
                            ops = [opa[:], opb[:]]
                    else:
                        ops = None
                    for ui in range(2):
                        u0 = (upair * 2 + ui) * TS
                        usl = slice(u0, u0 + TS)
                        if fin:
                            op = ops[ui]
                        else:
                            op = opp.tile([P, TS], F32, tag="op",
                                          name="op")[:]
                        nc.tensor.matmul(op, aoh[:, :, trow],
                                         woh[:, :, usl],
                                         start=True, stop=False, perf_mode=DR)
                        nc.tensor.matmul(op, aoh[:, :, trow],
                                         wol[:, :, usl],
                                         start=False, stop=False, perf_mode=DR)
                        nc.tensor.matmul(op, aol[:, :, trow],
                                         woh[:, :, usl],
                                         start=False, stop=True, perf_mode=DR)
                        if ui == 0:
                            nc.vector.tensor_scalar(
                                ot[:, usl], op, 1.0 / (WSCL * AOSCL),
                                None, ALU.mult)
                        else:
                            nc.scalar.activation(
                                ot[:, usl], op, AF.Copy,
                                scale=1.0 / (WSCL * AOSCL))
                    if fin:
                        # finer store granularity so the kernel-exit drain
                        # waits on a small final DMA
                        uhalf = slice(upair * 2 * TS, (upair + 1) * 2 * TS)
                        nc.sync.dma_start(o_dram[trow, uhalf], ot[:, uhalf])
                if not fin:
                    nc.sync.dma_start(o_dram[trow, :], ot[:])

        # Emission order: A(sl+1)'s jobs are emitted interleaved between
        # B(sl)'s heads so the list scheduler sees projection filler near
        # the attention stalls it should fill. cos/sin(0) and x(0) were
        # loaded by the chunked startup loads above.
        emit_A_jobs(0, [("q", 0), ("q", 1), ("k", 0), ("v", 0)])
        for sl in range(NSL):
            if sl + 1 < NSL:
                load_x_cs(sl + 1)
            if sl == 0:
                nc.sync.dma_start(woh[:], io["woth"][:])
                nc.sync.dma_start(wol[:], io["wotl"][:])
            emit_B_head(sl, 0)
            if sl + 1 < NSL:
                # q0 and k first: B(sl+1) head 0 needs them; q1 can finish
                # during B(sl+1, 0)
                emit_A_jobs(sl + 1, [("q", 0), ("k", 0)])
            elif sl == NSL - 1:
                # deferred C(NSL-2): the only PE filler left for the last
                # slice's exp-paced attention (A is done by now)
                emit_C(NSL - 2)
            emit_B_head(sl, 1)
            if sl + 1 < NSL:
                emit_A_jobs(sl + 1, [("v", 0), ("q", 1)])
            if sl < NSL - 2 or sl == NSL - 1:
                emit_C(sl)


def _build_nc():
    nc = bacc.Bacc("TRN2", target_bir_lowering=False, debug=False,
                   enable_asserts=False, num_devices=NCORES)
    io = {}

    def din(name, shape, dt):
        io[name] = nc.dram_tensor(name, shape, dt, kind="ExternalInput").ap()

    din("xts", [NSL, P, HO, 2, TS], F8)      # x^T hi|lo interleaved
    din("wqth", [P, HO, HPC * D], F8)        # 64*wq hi
    din("wqtl", [P, HO, HPC * D], F8)
    din("wkth", [P, HO, D], F8)
    din("wktl", [P, HO, D], F8)
    din("wvth", [P, HO, D], F8)
    din("wvtl", [P, HO, D], F8)
    din("woth", [P, HPC, HID], F8)
    din("wotl", [P, HPC, HID], F8)
    din("cst", [P, 2, T], BF16)              # [cos; sin], transposed
    din("cb32", [P, 4 + P], F32)             # [bq | bk | bv | iden]
    din("cb16", [P, 3 * P], BF16)            # [rot | ones/8 | tri]
    o = nc.dram_tensor("o_part", [T, HID], F16, kind="ExternalOutput").ap()
    _emit(nc, io, o)
    nc.compile()
    return nc


def _get_nc():
    if "nc" not in _CACHE:
        _CACHE["nc"] = _build_nc()
    return _CACHE["nc"]


def _split8(a):
    hi = a.astype(NPF8)
    lo = (a - hi.astype(np.float32)).astype(NPF8)
    return hi, lo


def _consts():
    """cb16 blob [P, 3P] = [rot | ones/8 | tri] in bf16."""
    if "consts" in _CACHE:
        return _CACHE["consts"]
    # rotate_half as a matmul on lin: out[d,t] = sum_e R[e,d] lin[e,t]
    R = np.zeros((P, P), np.float32)
    for e in range(P // 2):
        R[e, e + P // 2] = 1.0      # d >= 64 takes +q[d-64]
    for e in range(P // 2, P):
        R[e, e - P // 2] = -1.0     # d < 64 takes -q[d+64]
    onesm = np.full((P, P), 1.0 / AOSCL, np.float32)
    tri = np.triu(np.ones((P, P), np.float32))
    _CACHE["consts"] = np.ascontiguousarray(
        np.concatenate([R, onesm, tri], axis=1)).astype(NPBF16)
    return _CACHE["consts"]


def kernel(x, cos, sin, wq, bq, wk, bk, wv, bv, wo):
    x = np.asarray(x, dtype=np.float32)
    cos = np.asarray(cos, dtype=np.float32)
    sin = np.asarray(sin, dtype=np.float32)
    wq = np.asarray(wq, dtype=np.float32)
    bq = np.asarray(bq, dtype=np.float32)
    wk = np.asarray(wk, dtype=np.float32)
    bk = np.asarray(bk, dtype=np.float32)
    wv = np.asarray(wv, dtype=np.float32)
    bv = np.asarray(bv, dtype=np.float32)
    wo = np.asarray(wo, dtype=np.float32)

    nc = _get_nc()
    cb16 = _consts()

    # x^T tiled: xtt[sl, p, ho, c] = x[0, sl*TS + c, ho*P + p]; fp8 hi/lo
    xT = np.ascontiguousarray(x[0].T)                             # [HID, T]
    xtt = np.ascontiguousarray(
        xT.reshape(HO, P, NSL, TS).transpose(2, 1, 0, 3))         # [NSL,P,HO,TS]
    xtth, xttl = _split8(xtt)
    xts = np.ascontiguousarray(
        np.stack([xtth, xttl], axis=3))                   # [NSL,P,HO,2,TS]

    cst = np.ascontiguousarray(
        np.stack([cos.T, sin.T], axis=1)).astype(NPBF16)          # [P, 2, T]

    def wtile(wslice):  # [J, HID] -> [P, HO, J] with h = ho*P + p
        J = wslice.shape[0]
        return np.ascontiguousarray(
            wslice.T.reshape(HO, P, J).transpose(1, 0, 2)) * WSCL

    in_maps = []
    for c in range(NCORES):
        j0 = c * HPC * D
        kvh = c // (NCORES // NKV)
        wqth, wqtl = _split8(wtile(wq[j0:j0 + HPC * D]))          # [P, HO, 256]
        wkth, wktl = _split8(wtile(wk[kvh * D:(kvh + 1) * D]))    # [P, HO, 128]
        wvth, wvtl = _split8(wtile(wv[kvh * D:(kvh + 1) * D]))
        # woT: [P, HPC, HID] with j = h*P + p
        wot = np.ascontiguousarray(
            wo[:, j0:j0 + HPC * D].T.reshape(HPC, P, HID)
            .transpose(1, 0, 2)) * WSCL
        woth, wotl = _split8(wot)
        bqt = np.ascontiguousarray(bq[j0:j0 + HPC * D].reshape(HPC, P).T)
        bkt = bk[kvh * D:(kvh + 1) * D].reshape(P, 1)
        bvt = bv[kvh * D:(kvh + 1) * D].reshape(P, 1)
        cb32 = np.concatenate(
            [bqt, bkt, bvt, np.eye(P)], axis=1).astype(np.float32)
        in_maps.append({
            "xts": xts,
            "wqth": wqth, "wqtl": wqtl, "wkth": wkth, "wktl": wktl,
            "wvth": wvth, "wvtl": wvtl, "woth": woth, "wotl": wotl,
            "cst": cst, "cb32": cb32, "cb16": cb16,
        })

    res = run_bass_kernel_spmd(nc, in_maps, list(range(NCORES)))
    out = np.zeros((T, HID), np.float32)
    for c in range(NCORES):
        out += res.results[c]["o_part"].astype(np.float32)
    return out.reshape(1, T, HID)
